# revision 1
# baseline (speedup 1.0000x reference)
"""ChebConv GNN kernel for Trainium2 (8 NeuronCores, data-parallel over batch).

The reference network (per graph, N=24 nodes):
  h1 = elu(sum_k Tk(L) x Wk + b1)     ChebConv K=3, 4->8
  h2 = elu(sum_k Tk(L) h1 Wk + b2)    ChebConv K=3, 8->8
  out = log_softmax(fc2(fc1(h2.flat)))

Everything is linear between the two ELUs and the final log_softmax, and the
Chebyshev propagation matrices A_k (24x24) are batch-independent.  The whole
network collapses to a per-graph MLP:
  z  = x.reshape(96)
  p1 = M1.T @ z  + b1          M1:[96,192]  = sum_k A_k (x) W1_k
  h1 = elu(p1)
  p2 = M2.T @ h1 + c2          M2:[192,192]
  h2 = elu(p2)
  d  = wd . h2   + bd2         wd:[192]  (fc2@fc1 fused, logit difference)
  out = [-softplus(d), -softplus(-d)]
M1/M2/wd are precomputed on host from edge_index + weights (all tiny).

On-chip: activations are feature-major [features, batch_cols]; matmuls run in
float32r (TF32-class, full PE rate at N=512).  ELU is a SINGLE ScalarE pass:
we compile with a patched activation table where Exp's positive-side spline
buckets compute x+1, so ActivationFunctionType.Exp evaluates elu(x)+1 exactly
(negative side is e^x, already elu(x)+1 there).  The +1 shift is absorbed in
the next layer's bias; per-feature biases ride the ACT bias port.
softplus(d) = relu(d) + Ln(Exp(-|d|) + 1) still works because Exp is
unchanged for arguments <= 0.  Tiles are processed in pairs so each ELU is
one wide ACT op ([128,1024] for the 128-feature block; the two 64-feature
blocks share one [128,512] psum bank via tile_position col packing).
"""

import json
import os
import shutil
import sys
import tempfile

import numpy as np

sys.path.insert(0, "/opt/trn_rl_repo")

B, N, F_IN, HID, K, NCLS = 131072, 24, 4, 8, 3, 2
FC1 = 64
NCORES = 8
R = B // NCORES          # rows (graphs) per core = 16384
TCOLS = 512              # graphs per tile
NTILES = R // TCOLS      # 32
FIN = N * F_IN           # 96
FH = N * HID             # 192

_ACT_SET = "natural_log_exp_and_others"


def _prepare_act_tables() -> str:
    """Copy the stock activation tables and patch Exp's positive-side
    buckets from e^x to x+1, turning Exp into elu(x)+1.  Returns the
    path to the patched act_info.json."""
    dst = os.path.join(tempfile.gettempdir(), "bass_elu_act_tables_v1")
    marker = os.path.join(dst, ".patched_ok")
    if os.path.exists(marker):
        return os.path.join(dst, "act_info.json")

    from neuronxcc.driver.Job import Job
    from neuronxcc.driver.jobs.support.FindActInfo import findActInfoFile

    src = os.path.dirname(findActInfoFile(Job.getPackageDir(), "gen3"))
    if os.path.exists(dst):
        shutil.rmtree(dst)
    shutil.copytree(src, dst)
    for root, _, files in os.walk(dst):
        for f in files:
            os.chmod(os.path.join(root, f), 0o644)

    with open(os.path.join(dst, f"{_ACT_SET}.json")) as f:
        prof = json.load(f)
    b0 = prof["func_to_bkt_start_idx"]["exp"]
    starts = sorted(prof["func_to_bkt_start_idx"].values())
    b1 = min(s for s in starts if s > b0)

    path = os.path.join(dst, f"{_ACT_SET}_bkt.bin")
    raw = np.fromfile(path, dtype=np.float32).reshape(-1, 8).copy()
    for i in range(b0, b1):
        x0 = raw[i, 4]
        if x0 > 0.0:
            raw[i, :5] = [x0 + 1.0, 1.0, 0.0, 0.0, x0]
    raw.tofile(path)
    with open(marker, "w") as f:
        f.write("ok")
    return os.path.join(dst, "act_info.json")


def _install_ntff_hook():
    """Register the axon NTFF-profiling hook that the agent image's antenv
    package lacks, so run_bass_kernel_spmd(trace=True) can capture HW
    profiles through the tunnel."""
    if "antenv.axon_hooks" in sys.modules:
        return True
    try:
        import types

        from trn_agent_boot.trn_boot import _ntff_profile_via_ctypes

        hook = _ntff_profile_via_ctypes("/opt/axon/libaxon_pjrt.so")
        mod = types.ModuleType("antenv.axon_hooks")
        mod.get_axon_ntff_profile_hook = lambda: hook
        mod.set_axon_ntff_profile_hook = lambda h: None
        sys.modules["antenv.axon_hooks"] = mod
        return True
    except Exception as e:  # pragma: no cover - profiling is best-effort
        print("ntff hook install failed:", e)
        return False


def _patch_tile_drain():
    """This walrus build rejects TPB_CTRL instructions with more than one
    sem wait; split the TileContext tail drain into one drain per wait."""
    import concourse.tile as tile_mod
    from concourse.vector_clock import ScopedClock, VectorClock

    if getattr(tile_mod.TileContext, "_drain_patched", False):
        return

    def _drain_and_barrier(self, tick_clock, wait_clock):
        gc = tick_clock.global_clock
        n = len(gc)
        for p in range(n):
            t = gc[p]
            if t <= 0:
                continue
            vec = [0] * n
            vec[p] = t
            d = self.nc.sync.drain()
            wait_clock.add_sem_waits(d.ins, ScopedClock({None: VectorClock(vec)}))
        self.nc.all_engine_barrier()
        popped = self.nc._tile_sem_poison_stack.pop()
        assert popped is self._sem_poison
        self.nc.clear_and_free_semaphores(list(self.sems.allocated().values()))
        self.nc.all_engine_barrier()

    tile_mod.TileContext._drain_and_barrier = _drain_and_barrier
    tile_mod.TileContext._drain_patched = True


def _split_multiwaits(nc):
    """This walrus build accepts at most one sem-wait per instruction.
    Post-process the serialized BIR: for every instruction carrying N>1
    waits, insert N-1 single-wait NoOp instructions just before it on the
    same engine."""
    orig = nc.to_json_bytes

    def patched():
        m = json.loads(orig())
        counter = [0]
        for func in m["functions"]:
            for blk in func["blocks"]:
                out = []
                for inst in blk["instructions"]:
                    si = inst.get("sync_info")
                    ow = (si or {}).get("on_wait") or []
                    eng = inst.get("engine", "Unassigned")
                    if len(ow) > 1 and eng != "Unassigned":
                        for w in ow[:-1]:
                            counter[0] += 1
                            out.append({
                                "debug": inst.get("debug", 0),
                                "engine": eng,
                                "ins": [],
                                "name": f"IWS-{counter[0]}",
                                "opcode": "NoOp",
                                "outs": [],
                                "sync_info": {"on_wait": [w]},
                            })
                        si["on_wait"] = [ow[-1]]
                    out.append(inst)
                blk["instructions"] = out
        return json.dumps(m).encode()

    nc.to_json_bytes = patched


def _host_weights(edge_index, conv1_W, conv1_b, conv2_W, conv2_b,
                  fc1_W, fc1_b, fc2_W, fc2_b):
    """Fold graph propagation + all linear layers into dense matrices."""
    ei = np.asarray(edge_index)
    row, col = ei[0].astype(np.int64), ei[1].astype(np.int64)
    deg = np.zeros(N, np.float64)
    np.add.at(deg, row, 1.0)
    dis = np.where(deg > 0, deg ** -0.5, 0.0)
    ew = -dis[row] * dis[col]
    S = np.zeros((N, N), np.float64)
    np.add.at(S, (row, col), ew)

    A = np.stack([np.eye(N), S, 2.0 * (S @ S) - np.eye(N)])  # [3,24,24]

    W1 = np.asarray(conv1_W, np.float64)   # [3,4,8]
    W2 = np.asarray(conv2_W, np.float64)   # [3,8,8]
    # M1[(m,f),(n,h)] = sum_k A_k[n,m] W1_k[f,h]
    M1 = np.einsum('knm,kfh->mfnh', A, W1).reshape(FIN, FH)
    M2 = np.einsum('knm,kgh->mgnh', A, W2).reshape(FH, FH)
    b1 = np.tile(np.asarray(conv1_b, np.float64), N)          # [192]
    b2 = np.tile(np.asarray(conv2_b, np.float64), N)          # [192]

    Wf = np.asarray(fc2_W, np.float64) @ np.asarray(fc1_W, np.float64)  # [2,192]
    bf = np.asarray(fc2_W, np.float64) @ np.asarray(fc1_b, np.float64) \
        + np.asarray(fc2_b, np.float64)                                  # [2]
    wd = Wf[1] - Wf[0]
    bd = bf[1] - bf[0]

    # ELU pass returns elu(y)+1; absorb the -1 into the consumer's bias.
    c2 = b2 - M2.sum(axis=0)       # bias for layer2 given h1' = h1+1
    bd2 = bd - wd.sum()            # bias for fc given h2' = h2+1

    return (M1.astype(np.float32), b1.astype(np.float32),
            M2.astype(np.float32), c2.astype(np.float32),
            wd.astype(np.float32), float(bd2))


def _build_bass():
    import concourse.bass as bass
    import concourse.mybir as mybir
    from concourse.tile import TileContext

    _patch_tile_drain()

    f32 = mybir.dt.float32
    f32r = mybir.dt.float32r
    AF = mybir.ActivationFunctionType
    ALU = mybir.AluOpType

    nc = bass.Bass(debug=False)

    xs = nc.dram_tensor("xs", [R, FIN], f32, kind="ExternalInput").ap()
    ident_d = nc.dram_tensor("ident", [128, 128], f32, kind="ExternalInput").ap()
    m1_d = nc.dram_tensor("m1", [FIN, FH], f32r, kind="ExternalInput").ap()
    m2a_d = nc.dram_tensor("m2a", [128, FH], f32r, kind="ExternalInput").ap()
    m2b_d = nc.dram_tensor("m2b", [64, FH], f32r, kind="ExternalInput").ap()
    wda_d = nc.dram_tensor("wda", [128, NTILES, 32], f32r,
                           kind="ExternalInput").ap()
    wdb_d = nc.dram_tensor("wdb", [64, NTILES, 32], f32r,
                           kind="ExternalInput").ap()
    # biases: [b1(0:128)], [b1(128:192) x2], [c2(0:128)], [c2(128:192) x2],
    # [bd2], [-bd2]
    bia_d = nc.dram_tensor("bia", [128, 6], f32, kind="ExternalInput").ap()
    out_d = nc.dram_tensor("out", [R, NCLS], f32, kind="ExternalOutput").ap()

    bd2 = float(np.float32(0.0))  # patched at build time via closure below

    with TileContext(nc) as tc:
        with (
            tc.tile_pool(name="consts", bufs=1) as cpool,
            tc.tile_pool(name="load", bufs=3) as lpool,
            tc.tile_pool(name="act", bufs=2) as apool,
            tc.tile_pool(name="tail", bufs=1) as tpool,
            # slotA: transpose pair-psum and L1-b pair (disjoint lifetimes)
            # slotC: L1-a pair and L2-a pair (disjoint lifetimes)
            tc.tile_pool(name="psa", bufs=1, space="PSUM") as psa,
            tc.tile_pool(name="psc", bufs=1, space="PSUM") as psc,
            tc.tile_pool(name="psb", bufs=1, space="PSUM") as psb,
            tc.tile_pool(name="pd", bufs=1, space="PSUM") as pd,
        ):
            ident = cpool.tile([128, 128], f32)
            nc.sync.dma_start(out=ident[:], in_=ident_d[:])
            m1 = cpool.tile([FIN, FH], f32r)
            nc.sync.dma_start(out=m1[:], in_=m1_d[:])
            m2a = cpool.tile([128, FH], f32r)
            nc.sync.dma_start(out=m2a[:], in_=m2a_d[:])
            m2b = cpool.tile([64, FH], f32r)
            nc.sync.dma_start(out=m2b[:], in_=m2b_d[:])
            wda = cpool.tile([128, NTILES, 32], f32r)
            nc.sync.dma_start(out=wda[:], in_=wda_d[:])
            wdb = cpool.tile([64, NTILES, 32], f32r)
            nc.sync.dma_start(out=wdb[:], in_=wdb_d[:])
            bia = cpool.tile([128, 6], f32)
            nc.sync.dma_start(out=bia[:], in_=bia_d[:])

            dstage = pd.tile([32, TCOLS], f32)   # fc logit-diffs, one row/tile

            for u in range(NTILES // 2):
                pT = psa.tile([FIN, 2 * TCOLS], f32, space="PSUM", tag="sa")
                zt = apool.tile([FIN, 2 * TCOLS], f32r)
                for k in range(2):
                    t = 2 * u + k
                    xb = lpool.tile([128, 4, FIN], f32)
                    src = xs[t * TCOLS:(t + 1) * TCOLS, :].rearrange(
                        "(g p) f -> p g f", p=128)
                    nc.sync.dma_start(out=xb[:], in_=src)
                    for g in range(4):
                        nc.tensor.transpose(
                            out=pT[:, k * TCOLS + g * 128:
                                   k * TCOLS + (g + 1) * 128],
                            in_=xb[:, g, :],
                            identity=ident[:],
                        )
                nc.vector.tensor_copy(out=zt[:], in_=pT[:])

                # ---- layer 1 ----
                p1a = psc.tile([128, 2 * TCOLS], f32, space="PSUM", tag="sc")
                p1b = psa.tile([64, 2 * TCOLS], f32, space="PSUM", tag="sa")
                for k in range(2):
                    ztk = zt[:, k * TCOLS:(k + 1) * TCOLS]
                    sl = slice(k * TCOLS, (k + 1) * TCOLS)
                    nc.tensor.matmul(out=p1a[:, sl], lhsT=m1[:, 0:128],
                                     rhs=ztk, start=True, stop=True)
                    nc.tensor.matmul(out=p1b[:, sl], lhsT=m1[:, 128:FH],
                                     rhs=ztk, start=True, stop=True)
                h1a = apool.tile([128, 2 * TCOLS], f32r)
                h1b = apool.tile([64, 2 * TCOLS], f32r)
                nc.scalar.activation(h1a[:], p1a[:], AF.Exp,
                                     bias=bia[:, 0:1])
                nc.scalar.activation(h1b[:], p1b[:], AF.Exp,
                                     bias=bia[0:64, 1:2])

                # ---- layer 2 ----
                p2a = psc.tile([128, 2 * TCOLS], f32, space="PSUM", tag="sc")
                p2b = psb.tile([64, 2 * TCOLS], f32, space="PSUM")
                for k in range(2):
                    sl = slice(k * TCOLS, (k + 1) * TCOLS)
                    ha = h1a[:, sl]
                    hb = h1b[:, sl]
                    nc.tensor.matmul(out=p2a[:, sl], lhsT=m2a[:, 0:128],
                                     rhs=ha, start=True, stop=False)
                    nc.tensor.matmul(out=p2a[:, sl], lhsT=m2b[:, 0:128],
                                     rhs=hb, start=False, stop=True)
                    nc.tensor.matmul(out=p2b[:, sl], lhsT=m2a[:, 128:FH],
                                     rhs=ha, start=True, stop=False)
                    nc.tensor.matmul(out=p2b[:, sl], lhsT=m2b[:, 128:FH],
                                     rhs=hb, start=False, stop=True)
                h2a = apool.tile([128, 2 * TCOLS], f32r)
                h2b = apool.tile([64, 2 * TCOLS], f32r)
                nc.scalar.activation(h2a[:], p2a[:], AF.Exp,
                                     bias=bia[:, 2:3])
                nc.scalar.activation(h2b[:], p2b[:], AF.Exp,
                                     bias=bia[0:64, 3:4])

                # ---- fc: d(tile t) lands in dstage row t ----
                for k in range(2):
                    t = 2 * u + k
                    sl = slice(k * TCOLS, (k + 1) * TCOLS)
                    nc.tensor.matmul(out=dstage[:], lhsT=wda[:, t, :],
                                     rhs=h2a[:, sl],
                                     start=(t == 0), stop=False,
                                     skip_group_check=True)
                    nc.tensor.matmul(out=dstage[:], lhsT=wdb[:, t, :],
                                     rhs=h2b[:, sl],
                                     start=False, stop=(t == NTILES - 1),
                                     skip_group_check=True)

            # ---- tail: out0 = -softplus(d'), out1 = -softplus(-d') ----
            # d' = d + bd2;  softplus(y) = relu(y) + ln(1 + e^-|y|)
            t1 = tpool.tile([32, TCOLS], f32)
            t2 = tpool.tile([32, TCOLS], f32)
            t3 = tpool.tile([32, TCOLS], f32)
            ra = tpool.tile([32, TCOLS], f32)
            rb = tpool.tile([32, TCOLS], f32)
            uu = tpool.tile([32, TCOLS], f32)
            v = tpool.tile([32, TCOLS, 2], f32)
            nc.scalar.activation(t1[:], dstage[:], AF.Abs,
                                 bias=bia[0:32, 4:5])
            nc.scalar.activation(t2[:], t1[:], AF.Exp, scale=-1.0)
            nc.scalar.activation(t3[:], t2[:], AF.Ln, bias=1.0)
            nc.scalar.activation(ra[:], dstage[:], AF.Relu,
                                 bias=bia[0:32, 4:5])
            nc.scalar.activation(rb[:], dstage[:], AF.Relu, scale=-1.0,
                                 bias=bia[0:32, 5:6])
            nc.vector.tensor_scalar_mul(uu[:], t3[:], -1.0)
            nc.vector.tensor_tensor(out=v[:, :, 0], in0=uu[:],
                                    in1=ra[:], op=ALU.subtract)
            nc.vector.tensor_tensor(out=v[:, :, 1], in0=uu[:],
                                    in1=rb[:], op=ALU.subtract)
            dst = out_d.rearrange("(p n) c -> p n c", p=32)
            nc.sync.dma_start(out=dst, in_=v[:])

    _split_multiwaits(nc)
    return nc


_BD2 = [0.0]


def _make_inputs(**inputs):
    M1, b1, M2, c2, wd, bd2 = _host_weights(
        inputs["edge_index"], inputs["conv1_W"], inputs["conv1_b"],
        inputs["conv2_W"], inputs["conv2_b"], inputs["fc1_W"],
        inputs["fc1_b"], inputs["fc2_W"], inputs["fc2_b"])
    _BD2[0] = float(bd2)

    m2b = M2[128:FH]                                         # [64,192]
    wda = np.zeros((128, NTILES, 32), np.float32)
    wdb = np.zeros((64, NTILES, 32), np.float32)
    for t in range(NTILES):
        j = t % 32
        wda[:, t, j] = wd[0:128]
        wdb[:, t, j] = wd[128:FH]
    bia = np.zeros((128, 6), np.float32)
    bia[:, 0] = b1[0:128]
    bia[0:64, 1] = b1[128:FH]
    bia[64:128, 1] = b1[128:FH]
    bia[:, 2] = c2[0:128]
    bia[0:64, 3] = c2[128:FH]
    bia[64:128, 3] = c2[128:FH]
    bia[:, 4] = bd2
    bia[:, 5] = -bd2
    ident = np.eye(128, dtype=np.float32)

    x = np.ascontiguousarray(np.asarray(inputs["x"], np.float32)).reshape(B, FIN)
    const = dict(ident=ident, m1=np.ascontiguousarray(M1),
                 m2a=np.ascontiguousarray(M2[0:128]),
                 m2b=np.ascontiguousarray(m2b),
                 wda=wda, wdb=wdb, bia=bia)
    in_maps = []
    for c in range(NCORES):
        m = dict(const)
        m["xs"] = np.ascontiguousarray(x[c * R:(c + 1) * R])
        in_maps.append(m)
    return in_maps


_LAST_RESULTS = {}


def kernel(**inputs) -> np.ndarray:
    os.environ["BASS_ACT_ROOT_JSON_PATH"] = _prepare_act_tables()
    os.environ["NEURON_FORCE_RECOMPILE"] = "1"

    from concourse.bass_utils import run_bass_kernel_spmd

    in_maps = _make_inputs(**inputs)   # sets _BD2 before building
    nc = _build_bass()
    trace = os.environ.get("KERNEL_TRACE", "0") == "1"
    if trace:
        trace = _install_ntff_hook()
    res = run_bass_kernel_spmd(
        nc, in_maps, core_ids=list(range(NCORES)), trace=trace,
        stitch_traces=False,
    )
    _LAST_RESULTS["exec_time_ns"] = res.exec_time_ns
    _LAST_RESULTS["mean_exec_time_ns"] = res.mean_exec_time_ns
    _LAST_RESULTS["trace"] = res.instructions_and_trace
    out = np.concatenate([r["out"] for r in res.results], axis=0)
    return out.reshape(B, 1, NCLS)

